# revision 10
# baseline (speedup 1.0000x reference)
"""LocallyConnectedXYZLayer Trainium2 kernel.

out[n,c,i,j] = sum_{dh,dw in 5x5} sm[n,c,i+dh,(j+dw)%W] * mask[...] *
               exp(-||xyz[:,i+dh,(j+dw)%W] - xyz[:,i,j]||^2 / 2)
(zero-padded in H, circular in W)

Factorization used on device:
  exp(-d2/2) = exp(cross) * phi_src * phi_ctr,  phi = exp(-|xyz|^2/2),
  cross = x_s*x_c + y_s*y_c + z_s*z_c
so   out = phi_ctr * sum_k  psi_s[c] * exp(cross_k),
     psi[c] = sm[c] * mask * phi       (all per-pixel maps)

Sharding: 8 cores, each takes the full N=2 x H=64 rows (interleaved on the
128 SBUF partitions as p = i*2 + n so dh row-shifts are partition shifts
that never cross batches) and a 256-column W chunk with +-2 halo (circular).

The run is dominated by the axon tunnel (~25-55 MB/s per direction), so
I/O is minimized: xyz ships as fp16 and softmax as 6-bit (the {0,1} mask
and the round(sm*63) quantization are pre-folded on the host; 4 channels
pack into 3 bytes, grouped along C so masked pixels stay zero-byte runs
for the wire compression), all in a single u8 input tensor; the output
ships as uint8 with a dynamic per-partition scale packed into the same
tensor (4 f32 bytes per row).
The donated zero output buffers of the stock run_bass_kernel_spmd path
are dropped (the kernel writes every output element), the jitted
executable is cached across calls, and the per-shard D2H copies are
kicked off async so dequant/unshard overlaps the remaining transfers.

The 25-offset channel MAC runs on the vector engine with fp16 psi, f32
exp(cross), and an f32 accumulator (psi stored twice at even alignment so
every dw window read stays 4B-aligned for 16-bit mode); device exec is a
negligible share of the call, so precision is free.
"""

import sys

sys.path.insert(0, "/opt/trn_rl_repo")

import numpy as np

N, C, H, W = 2, 20, 64, 2048
NCORES = 8
WC = W // NCORES          # 256 columns per core
WH = WC + 4               # with halo
P = H * N                 # 128 partitions
FS = C * WC               # 5120 output values per row
FS6 = FS * 3 // 4         # 3840 bytes after 6-bit packing
NQ = FS // 4              # 1280 value-quads per row
OSCALE = 62.99            # quant scale: acc*s + 0.5 stays < 63.5
CHUNKS = ((0, 2), (2, 5), (5, 8))   # pipelined core groups

_CACHE = {}


def _build():
    import concourse.bass as bass
    import concourse.mybir as mybir
    from concourse.tile import TileContext
    from concourse import tile as tile_mod
    from concourse.vector_clock import ScopedClock

    # --- walrus in this env rejects >2 sem-waits on one CTRL inst: put the
    # final-drain waits on a chain of nops (2 waits each) instead.
    def _patched_dab(self, tick_clock, wait_clock):
        nc = self.nc
        carrier = nc.sync.nop(nofuse=True, hint="drain_waits")
        wait_clock.add_sem_waits(
            carrier.ins, ScopedClock({None: tick_clock.global_clock})
        )
        si = carrier.ins.sync_info
        if si is not None and len(si.on_wait) > 2:
            waits = list(si.on_wait)
            carrier.ins.sync_info = mybir.SyncInfo(
                on_wait=waits[:2], on_update=list(si.on_update)
            )
            rest = waits[2:]
            while rest:
                chunk, rest = rest[:2], rest[2:]
                extra = nc.sync.nop(nofuse=True, hint="drain_waits")
                extra.ins.sync_info = mybir.SyncInfo(on_wait=chunk, on_update=[])
        nc.sync.drain()
        nc.all_engine_barrier()
        popped = nc._tile_sem_poison_stack.pop()
        assert popped is self._sem_poison
        nc.clear_and_free_semaphores(list(self.sems.allocated().values()))
        nc.all_engine_barrier()

    tile_mod.TileContext._drain_and_barrier = _patched_dab

    def split_excess_waits(nc, max_waits=1):
        for f in nc.m.functions:
            for blk in f.blocks:
                insts = blk.instructions
                i = 0
                while i < len(insts):
                    inst = insts[i]
                    si = inst.sync_info
                    if si is not None and len(si.on_wait) > max_waits:
                        waits = list(si.on_wait)
                        keep = waits[:max_waits]
                        extra = waits[max_waits:]
                        k = 0
                        while extra:
                            chunk = extra[:max_waits]
                            extra = extra[max_waits:]
                            nop = mybir.InstNoOp(
                                name=f"{inst.name}_ws{k}",
                                engine=inst.engine, ins=[], outs=[],
                                sync_info=mybir.SyncInfo(on_wait=chunk,
                                                         on_update=[]),
                            )
                            insts.insert(i, nop)
                            i += 1
                            k += 1
                        inst.sync_info = mybir.SyncInfo(
                            on_wait=keep, on_update=list(si.on_update))
                    i += 1

    f32 = mybir.dt.float32
    f16 = mybir.dt.float16
    u8 = mybir.dt.uint8
    mult = mybir.AluOpType.mult
    add = mybir.AluOpType.add
    mx = mybir.AluOpType.max
    Exp = mybir.ActivationFunctionType.Exp
    Square = mybir.ActivationFunctionType.Square
    Copy = mybir.ActivationFunctionType.Copy

    nc = bass.Bass("TRN2", target_bir_lowering=False, debug=False,
                   num_devices=NCORES)
    AND = mybir.AluOpType.bitwise_and
    OR = mybir.AluOpType.bitwise_or
    SHL = mybir.AluOpType.logical_shift_left
    SHR = mybir.AluOpType.logical_shift_right

    # one packed input / one packed output to minimize axon round trips:
    # cin = [xyz as f16 bytes | sm63 packed 4 channels -> 3 bytes],
    # oout = [q u8 | scale f32 bytes].  The 6-bit groups run along C (4
    # channels of one pixel) so a masked pixel still yields 3-byte zero
    # runs that the H2D wire compression can eat.
    XB = 2 * 3 * WH                     # 1560 bytes of f16 coords
    CQ = C // 4                         # 5 channel-quads
    SB = CQ * 3 * WH                    # 3900 packed softmax bytes
    cin = nc.declare_dram_parameter("cin", [P, XB + SB], u8, isOutput=False)
    oout = nc.declare_dram_parameter("oout", [P, FS6 + 4], u8, isOutput=True)

    def view(t, poff, pc, off, dims):
        a = t[:]
        pstride = a.ap[0][0]
        return bass.AP(a.tensor, a.offset + poff * pstride + off,
                       [[pstride, pc]] + dims)

    with TileContext(nc) as tc:
        with tc.tile_pool(name="main", bufs=1) as pool, \
             tc.tile_pool(name="cross", bufs=2) as cpool, \
             tc.tile_pool(name="tmps", bufs=2) as tpool, \
             tc.tile_pool(name="shift", bufs=1) as spool:
            xt_b = pool.tile([P, XB], u8)
            nc.sync.dma_start(out=xt_b[:], in_=cin[:, 0:XB])
            smp = pool.tile([P, SB], u8)
            nc.sync.dma_start(out=smp[:], in_=cin[:, XB:XB + SB])
            # unpack 3 bytes -> 4 channels of 6-bit sm values (per pixel j,
            # channel-quad cq; little-endian 24-bit groups)
            smt_q = pool.tile([P, C * WH], u8)
            tub = pool.tile([P, WH], u8)
            # pixel-major packed layout: byte (j, cq, t) at j*15 + cq*3 + t
            # so a masked pixel is a 15-byte zero run (wire compression)
            for cq in range(CQ):
                b = [view(smp, 0, P, cq * 3 + t, [[15, WH]])
                     for t in range(3)]
                v = [view(smt_q, 0, P, (4 * cq + m) * WH, [[1, WH]])
                     for m in range(4)]
                t_ = tub[:]
                nc.vector.tensor_scalar(v[0], b[0], 63, None, AND)
                nc.vector.tensor_scalar(t_, b[1], 15, 2, AND, SHL)
                nc.vector.tensor_scalar(v[1], b[0], 6, None, SHR)
                nc.vector.tensor_tensor(v[1], v[1], t_, OR)
                nc.vector.tensor_scalar(t_, b[2], 3, 4, AND, SHL)
                nc.vector.tensor_scalar(v[2], b[1], 4, None, SHR)
                nc.vector.tensor_tensor(v[2], v[2], t_, OR)
                nc.vector.tensor_scalar(v[3], b[2], 2, None, SHR)
            # u8 -> fp16 (values 0..63 exact; the /63 dequant and the host
            # quant scale are both folded into the host-side final divide)
            smt_h = pool.tile([P, C * WH], f16)
            nc.scalar.copy(smt_h[:], smt_q[:])

            # fp16 (bitcast view of the u8 bytes) -> f32 coords
            xt = pool.tile([P, 3 * WH], f32)
            nc.scalar.copy(xt[:], xt_b[:].bitcast(f16))

            # q = x^2+y^2+z^2 -> phi = exp(-q/2)
            sq0 = pool.tile([P, WH], f32)
            sq1 = pool.tile([P, WH], f32)
            nc.scalar.activation(sq0[:], xt[:, 0:WH], Square)
            nc.scalar.activation(sq1[:], xt[:, WH:2 * WH], Square)
            nc.vector.tensor_add(sq0[:], sq0[:], sq1[:])
            nc.scalar.activation(sq1[:], xt[:, 2 * WH:3 * WH], Square)
            nc.vector.tensor_add(sq0[:], sq0[:], sq1[:])
            phi = pool.tile([P, WH], f32)
            nc.scalar.activation(phi[:], sq0[:], Exp, scale=-0.5)

            # psi[c] = sm255[c] * phi (mask pre-folded into sm on host; the
            # x255 scale rides through to the dynamic output scale), stored
            # twice in fp16: psiA at column parity 0, psiB pre-shifted by one
            # column, so dw in {0,2,4} reads psiA and dw in {1,3} reads psiB
            # at even element offsets (4B-aligned for DVE 2x mode).
            psiA = pool.tile([P, C * WH], f16)
            psiB = pool.tile([P, C * WH], f16)
            phi_bc = view(phi, 0, P, 0, [[0, C], [1, WH]])
            smt_v = view(smt_h, 0, P, 0, [[WH, C], [1, WH]])
            nc.vector.tensor_tensor(
                view(psiA, 0, P, 0, [[WH, C], [1, WH]]), smt_v, phi_bc, mult)
            # psiB[., c, j] = psiA[., c, j+1]; DMA has no alignment limits
            nc.sync.dma_start(
                out=view(psiB, 0, P, 0, [[WH, C], [1, WH - 1]]),
                in_=view(psiA, 0, P, 1, [[WH, C], [1, WH - 1]]))

            accV = pool.tile([P, FS], f32)    # f32 accumulator chain

            for dh in (0, -1, 1, -2, 2):
                pc = P - 2 * abs(dh)
                pi = max(0, 2 * dh)    # source partition offset
                po = max(0, -2 * dh)   # dest partition offset
                if dh == 0:
                    pA, pB, xs_t = psiA, psiB, xt
                else:
                    # row-shifted copies via DMA (engines cannot start an AP
                    # at partition % 32 != 0); memset first so the out-of-
                    # range rows read as zero.
                    pA = spool.tile([P, C * WH], f16, tag="pA")
                    pB = spool.tile([P, C * WH], f16, tag="pB")
                    xs_t = spool.tile([P, 3 * WH], f32, tag="xs")
                    nc.vector.memset(pA[:], 0.0)
                    nc.vector.memset(pB[:], 0.0)
                    nc.vector.memset(xs_t[:], 0.0)
                    nc.sync.dma_start(out=pA[po:po + pc, :],
                                      in_=psiA[pi:pi + pc, :])
                    nc.sync.dma_start(out=pB[po:po + pc, :],
                                      in_=psiB[pi:pi + pc, :])
                    nc.sync.dma_start(out=xs_t[po:po + pc, :],
                                      in_=xt[pi:pi + pc, :])
                # cross terms for all 5 dw at once: [P, 5, 256] f32
                m1 = cpool.tile([P, 5 * WC], f32, tag="m1")
                m2 = cpool.tile([P, 5 * WC], f32, tag="m2")
                m3 = cpool.tile([P, 5 * WC], f32, tag="m3")
                for d, mm in enumerate((m1, m2, m3)):
                    xs = view(xs_t, 0, P, d * WH, [[1, 5], [1, WC]])
                    xc = view(xt, 0, P, d * WH + 2, [[0, 5], [1, WC]])
                    mo = view(mm, 0, P, 0, [[WC, 5], [1, WC]])
                    nc.vector.tensor_tensor(mo, xs, xc, mult)
                v1 = view(m1, 0, P, 0, [[WC, 5], [1, WC]])
                v2 = view(m2, 0, P, 0, [[WC, 5], [1, WC]])
                v3 = view(m3, 0, P, 0, [[WC, 5], [1, WC]])
                nc.vector.tensor_tensor(v1, v1, v2, add)
                nc.vector.tensor_tensor(v1, v1, v3, add)
                ee = cpool.tile([P, 5 * WC], f32, tag="ee")
                ev = view(ee, 0, P, 0, [[WC, 5], [1, WC]])
                nc.scalar.activation(ev, v1, Exp)

                for dw in range(5):
                    src_t = pA if dw % 2 == 0 else pB
                    soff = dw if dw % 2 == 0 else dw - 1
                    ps = view(src_t, 0, P, soff, [[WH, C], [1, WC]])
                    eb = view(ee, 0, P, dw * WC, [[0, C], [1, WC]])
                    av = view(accV, 0, P, 0, [[WC, C], [1, WC]])
                    if dh == 0 and dw == 0:
                        nc.vector.tensor_tensor(av, ps, eb, mult)
                        continue
                    tmp = tpool.tile([P, FS], f32, tag="tmp")
                    tv = view(tmp, 0, P, 0, [[WC, C], [1, WC]])
                    nc.vector.tensor_tensor(tv, ps, eb, mult)
                    nc.vector.tensor_tensor(av, av, tv, add)

            # scale by phi_center in place, then quantize to u8 with a
            # per-partition dynamic scale (packed into the output bytes).
            ov = view(accV, 0, P, 0, [[WC, C], [1, WC]])
            pb = view(phi, 0, P, 2, [[0, C], [1, WC]])
            nc.vector.tensor_tensor(ov, ov, pb, mult)
            tmax = pool.tile([P, 1], f32)
            nc.vector.tensor_reduce(tmax[:], accV[:], mybir.AxisListType.X,
                                    mx)
            nc.vector.tensor_scalar_max(tmax[:], tmax[:], 1e-30)
            nc.sync.dma_start(out=oout[:, FS6:FS6 + 4],
                              in_=tmax[:].bitcast(u8))
            trec = pool.tile([P, 1], f32)
            nc.vector.reciprocal(trec[:], tmax[:])
            tsc = pool.tile([P, 1], f32)
            # OSCALE so v*s + 0.5 can never reach 63.5 (6-bit codes)
            nc.vector.tensor_scalar_mul(tsc[:], trec[:], OSCALE)
            out_q = pool.tile([P, FS], u8)
            nc.scalar.activation(out_q[:], accV[:], Copy, bias=0.0,
                                 scale=tsc[:])
            # pack 4 consecutive 6-bit codes -> 3 bytes (little-endian 24b)
            out_p = pool.tile([P, FS6], u8)
            qv = [view(out_q, 0, P, m, [[4, NQ]]) for m in range(4)]
            bv = [view(out_p, 0, P, t, [[3, NQ]]) for t in range(3)]
            tA = pool.tile([P, NQ], u8)
            tB = pool.tile([P, NQ], u8)
            nc.vector.tensor_scalar(tA[:], qv[1], 6, None, SHL)  # u8 wrap
            nc.vector.tensor_tensor(bv[0], qv[0], tA[:], OR)
            nc.vector.tensor_scalar(tA[:], qv[1], 2, None, SHR)
            nc.vector.tensor_scalar(tB[:], qv[2], 4, None, SHL)
            nc.vector.tensor_tensor(bv[1], tA[:], tB[:], OR)
            nc.vector.tensor_scalar(tA[:], qv[2], 4, None, SHR)
            nc.vector.tensor_scalar(tB[:], qv[3], 2, None, SHL)
            nc.vector.tensor_tensor(bv[2], tA[:], tB[:], OR)
            nc.sync.dma_start(out=oout[:, 0:FS6], in_=out_p[:])

    split_excess_waits(nc)
    return nc


def _get_runner():
    """Build nc + the jitted SPMD executor once; cache for warm calls."""
    if "runner" in _CACHE:
        return _CACHE["runner"]
    import jax
    from jax.sharding import Mesh, PartitionSpec
    from jax.experimental.shard_map import shard_map
    from concourse import bass2jax
    import concourse.mybir as mybir

    nc = _build()
    bass2jax.install_neuronx_cc_hook()
    partition_name = (nc.partition_id_tensor.name
                      if nc.partition_id_tensor else None)
    in_names, out_names, out_avals = [], [], []
    for alloc in nc.m.functions[0].allocations:
        if not isinstance(alloc, mybir.MemoryLocationSet):
            continue
        name = alloc.memorylocations[0].name
        if alloc.kind == "ExternalInput":
            if name != partition_name:
                in_names.append(name)
        elif alloc.kind == "ExternalOutput":
            out_names.append(name)
            out_avals.append(jax.core.ShapedArray(
                tuple(alloc.tensor_shape), mybir.dt.np(alloc.dtype)))
    bind_names = tuple(in_names) + ((partition_name,) if partition_name
                                    else ())

    def _body(*args):
        operands = list(args)
        if partition_name is not None:
            operands.append(bass2jax.partition_id_tensor())
        outs = bass2jax._bass_exec_p.bind(
            *operands,
            out_avals=tuple(out_avals),
            in_names=bind_names,
            out_names=tuple(out_names),
            lowering_input_output_aliases=(),
            sim_require_finite=True,
            sim_require_nnan=True,
            nc=nc,
        )
        return tuple(outs)

    devices = jax.devices()[:NCORES]
    fns = []
    for a, b in CHUNKS:
        mesh = Mesh(np.asarray(devices[a:b]), ("core",))
        fns.append(jax.jit(shard_map(
            _body, mesh=mesh,
            in_specs=(PartitionSpec("core"),) * len(in_names),
            out_specs=(PartitionSpec("core"),) * len(out_names),
            check_rep=False)))
    _CACHE["runner"] = (fns, in_names, out_names)
    return _CACHE["runner"]


XB = 2 * 3 * WH                 # bytes of f16 coords per row
CQ = C // 4                     # channel-quads for 6-bit packing
SB = CQ * 3 * WH                # packed softmax bytes per row
ROWB = XB + SB                  # bytes per cin row


def _get_prep_bufs():
    b = _CACHE.get("prep_bufs")
    if b is None:
        from numpy.lib.stride_tricks import as_strided
        cin = np.empty((NCORES * P, ROWB), np.uint8)
        b = _CACHE["prep_bufs"] = {
            "cin": cin,
            "f32": np.empty((N, C, H, W + 4), np.float32),
            "s_e": np.empty((N, C, H, W + 4), np.uint8),
            "x_e": np.empty((N, 3, H, W + 4), np.float16),
            # f16 / u8 views aliasing the packed buffer
            "xv": np.ndarray((NCORES, H, N, 3, WH), np.float16,
                             buffer=cin.data, offset=0,
                             strides=(H * N * ROWB, N * ROWB, ROWB,
                                      2 * WH, 2)),
            # pixel-major: byte (j, cq, t) at j*15 + cq*3 + t so masked
            # pixels are 15-byte zero runs (wire compression eats them)
            "svp": np.ndarray((NCORES, H, N, CQ, WH, 3), np.uint8,
                              buffer=cin.data, offset=XB,
                              strides=(H * N * ROWB, N * ROWB, ROWB,
                                       3, 15, 1)),
        }

        def win_view(a_e):  # (N, CD, H, W+4) -> (8, H, N, CD, WH) view
            t = a_e.transpose(2, 0, 1, 3)
            st = t.strides
            return as_strided(t, shape=(NCORES, H, N, a_e.shape[1], WH),
                              strides=(WC * st[3], st[0], st[1], st[2],
                                       st[3]))

        b["sviews"] = win_view(b["s_e"])
        b["xviews"] = win_view(b["x_e"])
    return b


def _prep_chunk(b, a_core, b_core, xyz, sm, mk):
    """Quantize/pack only the W-slice needed by cores [a_core, b_core).

    Straight serial numpy: this container has a single CPU core, so
    thread pools only add churn.
    """
    s_e, x_e, buf = b["s_e"], b["x_e"], b["f32"]
    c0 = a_core * WC
    c1 = min(b_core * WC + 2, W)         # +2: right halo of the last core
    if a_core == 0:
        # left wrap halo: ext[...,0:2] = quantized core cols W-2..W
        bs = buf[..., 0:2]
        np.multiply(sm[..., W - 2:W], 63.0, out=bs)
        bs += 0.5
        np.copyto(s_e[..., 0:2], bs, casting="unsafe")
        s_e[..., 0:2] *= mk[..., W - 2:W]
        np.copyto(x_e[..., 0:2], xyz[..., W - 2:W], casting="unsafe")
    bs = buf[..., c0:c1]
    np.multiply(sm[..., c0:c1], 63.0, out=bs)
    bs += 0.5
    core = s_e[..., c0 + 2:c1 + 2]
    np.copyto(core, bs, casting="unsafe")   # f32 -> u8 truncation = astype
    core *= mk[..., c0:c1]
    np.copyto(x_e[..., c0 + 2:c1 + 2], xyz[..., c0:c1], casting="unsafe")
    if b_core == NCORES:
        # right wrap halo: ext cols W+2..W+4 = core cols 0..2 (from chunk 0)
        s_e[..., W + 2:] = s_e[..., 2:4]
        x_e[..., W + 2:] = x_e[..., 2:4]
    svp, xv = b["svp"], b["xv"]
    sviews, xviews = b["sviews"], b["xviews"]
    for k in range(a_core, b_core):
        np.copyto(xv[k], xviews[k])
        # 4 channels (one quad) -> little-endian 24-bit group -> 3 bytes,
        # byte-plane u8 math: the wrapping shifts supply the bit masks
        sv = sviews[k]
        v0, v1, v2, v3 = (sv[:, :, 0::4, :], sv[:, :, 1::4, :],
                          sv[:, :, 2::4, :], sv[:, :, 3::4, :])
        svp[k, ..., 0] = v0 | (v1 << 6)          # wrap == (v1 & 3) << 6
        svp[k, ..., 1] = (v1 >> 2) | (v2 << 4)   # wrap == (v2 & 15) << 4
        svp[k, ..., 2] = (v2 >> 4) | (v3 << 2)


def kernel(xyz, softmax, mask):
    fns, in_names, out_names = _get_runner()
    oi = out_names.index("oout")
    xyz = np.asarray(xyz, np.float32)
    sm = np.asarray(softmax, np.float32)
    mk = np.asarray(mask).astype(np.uint8)[:, None]            # (N,1,H,W)
    b = _get_prep_bufs()
    cin = b["cin"]
    # pipelined: prep + dispatch chunk k, then prep k+1 while k uploads;
    # fetch/dequant chunk k while later chunks are still in flight
    chunk_outs = []
    for ci, (a, bb) in enumerate(CHUNKS):
        _prep_chunk(b, a, bb, xyz, sm, mk)
        arrs = fns[ci](cin[a * P:bb * P])
        pk = arrs[oi]
        for s in pk.addressable_shards:
            s.data.copy_to_host_async()
        chunk_outs.append(pk)
    out = np.empty((N, C, H, W), np.float32)
    ub = _CACHE.get("unpack_buf")
    if ub is None:
        ub = _CACHE["unpack_buf"] = np.empty((P, NQ, 4), np.uint8)
    for ci, (a, bb) in enumerate(CHUNKS):
        shards = sorted(chunk_outs[ci].addressable_shards,
                        key=lambda s: s.index[0].start or 0)
        for kk, s in enumerate(shards):
            k = a + kk
            qk = np.asarray(s.data)                            # (P, FS6+4) u8
            mx = qk[:, FS6:].copy().view(np.float32)           # (P, 1)
            # unpack 3 bytes -> 4 six-bit codes
            q3 = qk[:, :FS6].reshape(P, NQ, 3)
            b0, b1, b2 = q3[..., 0], q3[..., 1], q3[..., 2]
            ub[..., 0] = b0 & 63
            ub[..., 1] = (b0 >> 6) | ((b1 & 15) << 2)
            ub[..., 2] = (b1 >> 4) | ((b2 & 3) << 4)
            ub[..., 3] = b2 >> 2
            # dequant: device acc = 63*out_true, q ~= acc * OSCALE/max + .5
            sc = (mx * (1.0 / (OSCALE * 63.0))).reshape(H, N)  # per (i, n)
            np.multiply(
                ub.reshape(P, FS).reshape(H, N, C, WC).transpose(1, 2, 0, 3),
                sc.transpose(1, 0)[:, None, :, None],
                out=out[:, :, :, k * WC:(k + 1) * WC],
                dtype=np.float32)
    return out



# revision 13
# speedup vs baseline: 1.1317x; 1.1317x over previous
"""LocallyConnectedXYZLayer Trainium2 kernel.

out[n,c,i,j] = sum_{dh,dw in 5x5} sm[n,c,i+dh,(j+dw)%W] * mask[...] *
               exp(-||xyz[:,i+dh,(j+dw)%W] - xyz[:,i,j]||^2 / 2)
(zero-padded in H, circular in W)

Factorization used on device:
  exp(-d2/2) = exp(cross) * phi_src * phi_ctr,  phi = exp(-|xyz|^2/2),
  cross = x_s*x_c + y_s*y_c + z_s*z_c
so   out = phi_ctr * sum_k  psi_s[c] * exp(cross_k),
     psi[c] = sm[c] * mask * phi       (all per-pixel maps)

Sharding: 8 cores, each takes the full N=2 x H=64 rows (interleaved on the
128 SBUF partitions as p = i*2 + n so dh row-shifts are partition shifts
that never cross batches) and a 256-column W chunk with +-2 halo (circular).

The run is dominated by the axon tunnel (~25-55 MB/s per direction), so
I/O is minimized: xyz ships as fp16 and softmax as 6-bit (the {0,1} mask
and the round(sm*63) quantization are pre-folded on the host; 4 channels
pack into 3 bytes, grouped along C so masked pixels stay zero-byte runs
for the wire compression), all in a single u8 input tensor; the output
ships as uint8 with a dynamic per-partition scale packed into the same
tensor (4 f32 bytes per row).
The donated zero output buffers of the stock run_bass_kernel_spmd path
are dropped (the kernel writes every output element), the jitted
executable is cached across calls, and the per-shard D2H copies are
kicked off async so dequant/unshard overlaps the remaining transfers.

The 25-offset channel MAC runs on the vector engine with fp16 psi, f32
exp(cross), and an f32 accumulator (psi stored twice at even alignment so
every dw window read stays 4B-aligned for 16-bit mode); device exec is a
negligible share of the call, so precision is free.
"""

import sys

sys.path.insert(0, "/opt/trn_rl_repo")

import numpy as np

N, C, H, W = 2, 20, 64, 2048
NCORES = 8
WC = W // NCORES          # 256 columns per core
WH = WC + 4               # with halo
P = H * N                 # 128 partitions
FS = C * WC               # 5120 output values per row
FS6 = FS * 3 // 4         # 3840 bytes after 6-bit packing
NQ = FS // 4              # 1280 value-quads per row
OSCALE = 62.99            # quant scale: acc*s + 0.5 stays < 63.5
CHUNKS = ((0, 2), (2, 5), (5, 8))   # pipelined core groups

_CACHE = {}


def _build():
    import concourse.bass as bass
    import concourse.mybir as mybir
    from concourse.tile import TileContext
    from concourse import tile as tile_mod
    from concourse.vector_clock import ScopedClock

    # --- walrus in this env rejects >2 sem-waits on one CTRL inst: put the
    # final-drain waits on a chain of nops (2 waits each) instead.
    def _patched_dab(self, tick_clock, wait_clock):
        nc = self.nc
        carrier = nc.sync.nop(nofuse=True, hint="drain_waits")
        wait_clock.add_sem_waits(
            carrier.ins, ScopedClock({None: tick_clock.global_clock})
        )
        si = carrier.ins.sync_info
        if si is not None and len(si.on_wait) > 2:
            waits = list(si.on_wait)
            carrier.ins.sync_info = mybir.SyncInfo(
                on_wait=waits[:2], on_update=list(si.on_update)
            )
            rest = waits[2:]
            while rest:
                chunk, rest = rest[:2], rest[2:]
                extra = nc.sync.nop(nofuse=True, hint="drain_waits")
                extra.ins.sync_info = mybir.SyncInfo(on_wait=chunk, on_update=[])
        nc.sync.drain()
        nc.all_engine_barrier()
        popped = nc._tile_sem_poison_stack.pop()
        assert popped is self._sem_poison
        nc.clear_and_free_semaphores(list(self.sems.allocated().values()))
        nc.all_engine_barrier()

    tile_mod.TileContext._drain_and_barrier = _patched_dab

    def split_excess_waits(nc, max_waits=1):
        for f in nc.m.functions:
            for blk in f.blocks:
                insts = blk.instructions
                i = 0
                while i < len(insts):
                    inst = insts[i]
                    si = inst.sync_info
                    if si is not None and len(si.on_wait) > max_waits:
                        waits = list(si.on_wait)
                        keep = waits[:max_waits]
                        extra = waits[max_waits:]
                        k = 0
                        while extra:
                            chunk = extra[:max_waits]
                            extra = extra[max_waits:]
                            nop = mybir.InstNoOp(
                                name=f"{inst.name}_ws{k}",
                                engine=inst.engine, ins=[], outs=[],
                                sync_info=mybir.SyncInfo(on_wait=chunk,
                                                         on_update=[]),
                            )
                            insts.insert(i, nop)
                            i += 1
                            k += 1
                        inst.sync_info = mybir.SyncInfo(
                            on_wait=keep, on_update=list(si.on_update))
                    i += 1

    f32 = mybir.dt.float32
    f16 = mybir.dt.float16
    u8 = mybir.dt.uint8
    mult = mybir.AluOpType.mult
    add = mybir.AluOpType.add
    mx = mybir.AluOpType.max
    Exp = mybir.ActivationFunctionType.Exp
    Square = mybir.ActivationFunctionType.Square
    Copy = mybir.ActivationFunctionType.Copy

    nc = bass.Bass("TRN2", target_bir_lowering=False, debug=False,
                   num_devices=NCORES)
    AND = mybir.AluOpType.bitwise_and
    OR = mybir.AluOpType.bitwise_or
    SHL = mybir.AluOpType.logical_shift_left
    SHR = mybir.AluOpType.logical_shift_right

    # one packed input / one packed output to minimize axon round trips:
    # cin = [xyz as f16 bytes | sm63 packed 4 channels -> 3 bytes],
    # oout = [q u8 | scale f32 bytes].  The 6-bit groups run along C (4
    # channels of one pixel) so a masked pixel still yields 3-byte zero
    # runs that the H2D wire compression can eat.
    XB = 2 * 3 * WH                     # 1560 bytes of f16 coords
    CQ = C // 4                         # 5 channel-quads
    SB = CQ * 3 * WH                    # 3900 packed softmax bytes
    cin = nc.declare_dram_parameter("cin", [P, XB + SB], u8, isOutput=False)
    oout = nc.declare_dram_parameter("oout", [P, FS6 + 4], u8, isOutput=True)

    def view(t, poff, pc, off, dims):
        a = t[:]
        pstride = a.ap[0][0]
        return bass.AP(a.tensor, a.offset + poff * pstride + off,
                       [[pstride, pc]] + dims)

    with TileContext(nc) as tc:
        with tc.tile_pool(name="main", bufs=1) as pool, \
             tc.tile_pool(name="cross", bufs=2) as cpool, \
             tc.tile_pool(name="tmps", bufs=2) as tpool, \
             tc.tile_pool(name="shift", bufs=1) as spool:
            xt_b = pool.tile([P, XB], u8)
            nc.sync.dma_start(out=xt_b[:], in_=cin[:, 0:XB])
            smp = pool.tile([P, SB], u8)
            nc.sync.dma_start(out=smp[:], in_=cin[:, XB:XB + SB])
            # unpack 3 bytes -> 4 channels of 6-bit sm values (per pixel j,
            # channel-quad cq; little-endian 24-bit groups)
            smt_q = pool.tile([P, C * WH], u8)
            tub = pool.tile([P, WH], u8)
            # pixel-major packed layout: byte (j, cq, t) at j*15 + cq*3 + t
            # so a masked pixel is a 15-byte zero run (wire compression)
            for cq in range(CQ):
                b = [view(smp, 0, P, cq * 3 + t, [[15, WH]])
                     for t in range(3)]
                v = [view(smt_q, 0, P, (4 * cq + m) * WH, [[1, WH]])
                     for m in range(4)]
                t_ = tub[:]
                nc.vector.tensor_scalar(v[0], b[0], 63, None, AND)
                nc.vector.tensor_scalar(t_, b[1], 15, 2, AND, SHL)
                nc.vector.tensor_scalar(v[1], b[0], 6, None, SHR)
                nc.vector.tensor_tensor(v[1], v[1], t_, OR)
                nc.vector.tensor_scalar(t_, b[2], 3, 4, AND, SHL)
                nc.vector.tensor_scalar(v[2], b[1], 4, None, SHR)
                nc.vector.tensor_tensor(v[2], v[2], t_, OR)
                nc.vector.tensor_scalar(v[3], b[2], 2, None, SHR)
            # u8 -> fp16 (values 0..63 exact; the /63 dequant and the host
            # quant scale are both folded into the host-side final divide)
            smt_h = pool.tile([P, C * WH], f16)
            nc.scalar.copy(smt_h[:], smt_q[:])

            # fp16 (bitcast view of the u8 bytes) -> f32 coords
            xt = pool.tile([P, 3 * WH], f32)
            nc.scalar.copy(xt[:], xt_b[:].bitcast(f16))

            # q = x^2+y^2+z^2 -> phi = exp(-q/2)
            sq0 = pool.tile([P, WH], f32)
            sq1 = pool.tile([P, WH], f32)
            nc.scalar.activation(sq0[:], xt[:, 0:WH], Square)
            nc.scalar.activation(sq1[:], xt[:, WH:2 * WH], Square)
            nc.vector.tensor_add(sq0[:], sq0[:], sq1[:])
            nc.scalar.activation(sq1[:], xt[:, 2 * WH:3 * WH], Square)
            nc.vector.tensor_add(sq0[:], sq0[:], sq1[:])
            phi = pool.tile([P, WH], f32)
            nc.scalar.activation(phi[:], sq0[:], Exp, scale=-0.5)

            # psi[c] = sm255[c] * phi (mask pre-folded into sm on host; the
            # x255 scale rides through to the dynamic output scale), stored
            # twice in fp16: psiA at column parity 0, psiB pre-shifted by one
            # column, so dw in {0,2,4} reads psiA and dw in {1,3} reads psiB
            # at even element offsets (4B-aligned for DVE 2x mode).
            psiA = pool.tile([P, C * WH], f16)
            psiB = pool.tile([P, C * WH], f16)
            phi_bc = view(phi, 0, P, 0, [[0, C], [1, WH]])
            smt_v = view(smt_h, 0, P, 0, [[WH, C], [1, WH]])
            nc.vector.tensor_tensor(
                view(psiA, 0, P, 0, [[WH, C], [1, WH]]), smt_v, phi_bc, mult)
            # psiB[., c, j] = psiA[., c, j+1]; DMA has no alignment limits
            nc.sync.dma_start(
                out=view(psiB, 0, P, 0, [[WH, C], [1, WH - 1]]),
                in_=view(psiA, 0, P, 1, [[WH, C], [1, WH - 1]]))

            accV = pool.tile([P, FS], f32)    # f32 accumulator chain

            for dh in (0, -1, 1, -2, 2):
                pc = P - 2 * abs(dh)
                pi = max(0, 2 * dh)    # source partition offset
                po = max(0, -2 * dh)   # dest partition offset
                if dh == 0:
                    pA, pB, xs_t = psiA, psiB, xt
                else:
                    # row-shifted copies via DMA (engines cannot start an AP
                    # at partition % 32 != 0); memset first so the out-of-
                    # range rows read as zero.
                    pA = spool.tile([P, C * WH], f16, tag="pA")
                    pB = spool.tile([P, C * WH], f16, tag="pB")
                    xs_t = spool.tile([P, 3 * WH], f32, tag="xs")
                    nc.vector.memset(pA[:], 0.0)
                    nc.vector.memset(pB[:], 0.0)
                    nc.vector.memset(xs_t[:], 0.0)
                    nc.sync.dma_start(out=pA[po:po + pc, :],
                                      in_=psiA[pi:pi + pc, :])
                    nc.sync.dma_start(out=pB[po:po + pc, :],
                                      in_=psiB[pi:pi + pc, :])
                    nc.sync.dma_start(out=xs_t[po:po + pc, :],
                                      in_=xt[pi:pi + pc, :])
                # cross terms for all 5 dw at once: [P, 5, 256] f32
                m1 = cpool.tile([P, 5 * WC], f32, tag="m1")
                m2 = cpool.tile([P, 5 * WC], f32, tag="m2")
                m3 = cpool.tile([P, 5 * WC], f32, tag="m3")
                for d, mm in enumerate((m1, m2, m3)):
                    xs = view(xs_t, 0, P, d * WH, [[1, 5], [1, WC]])
                    xc = view(xt, 0, P, d * WH + 2, [[0, 5], [1, WC]])
                    mo = view(mm, 0, P, 0, [[WC, 5], [1, WC]])
                    nc.vector.tensor_tensor(mo, xs, xc, mult)
                v1 = view(m1, 0, P, 0, [[WC, 5], [1, WC]])
                v2 = view(m2, 0, P, 0, [[WC, 5], [1, WC]])
                v3 = view(m3, 0, P, 0, [[WC, 5], [1, WC]])
                nc.vector.tensor_tensor(v1, v1, v2, add)
                nc.vector.tensor_tensor(v1, v1, v3, add)
                ee = cpool.tile([P, 5 * WC], f32, tag="ee")
                ev = view(ee, 0, P, 0, [[WC, 5], [1, WC]])
                nc.scalar.activation(ev, v1, Exp)

                for dw in range(5):
                    src_t = pA if dw % 2 == 0 else pB
                    soff = dw if dw % 2 == 0 else dw - 1
                    ps = view(src_t, 0, P, soff, [[WH, C], [1, WC]])
                    eb = view(ee, 0, P, dw * WC, [[0, C], [1, WC]])
                    av = view(accV, 0, P, 0, [[WC, C], [1, WC]])
                    if dh == 0 and dw == 0:
                        nc.vector.tensor_tensor(av, ps, eb, mult)
                        continue
                    tmp = tpool.tile([P, FS], f32, tag="tmp")
                    tv = view(tmp, 0, P, 0, [[WC, C], [1, WC]])
                    nc.vector.tensor_tensor(tv, ps, eb, mult)
                    nc.vector.tensor_tensor(av, av, tv, add)

            # scale by phi_center in place, then quantize to u8 with a
            # per-partition dynamic scale (packed into the output bytes).
            ov = view(accV, 0, P, 0, [[WC, C], [1, WC]])
            pb = view(phi, 0, P, 2, [[0, C], [1, WC]])
            nc.vector.tensor_tensor(ov, ov, pb, mult)
            tmax = pool.tile([P, 1], f32)
            nc.vector.tensor_reduce(tmax[:], accV[:], mybir.AxisListType.X,
                                    mx)
            nc.vector.tensor_scalar_max(tmax[:], tmax[:], 1e-30)
            nc.sync.dma_start(out=oout[:, FS6:FS6 + 4],
                              in_=tmax[:].bitcast(u8))
            trec = pool.tile([P, 1], f32)
            nc.vector.reciprocal(trec[:], tmax[:])
            tsc = pool.tile([P, 1], f32)
            # OSCALE so v*s + 0.5 can never reach 63.5 (6-bit codes)
            nc.vector.tensor_scalar_mul(tsc[:], trec[:], OSCALE)
            out_q = pool.tile([P, FS], u8)
            nc.scalar.activation(out_q[:], accV[:], Copy, bias=0.0,
                                 scale=tsc[:])
            # pack 4 consecutive 6-bit codes -> 3 bytes (little-endian 24b)
            out_p = pool.tile([P, FS6], u8)
            qv = [view(out_q, 0, P, m, [[4, NQ]]) for m in range(4)]
            bv = [view(out_p, 0, P, t, [[3, NQ]]) for t in range(3)]
            tA = pool.tile([P, NQ], u8)
            tB = pool.tile([P, NQ], u8)
            nc.vector.tensor_scalar(tA[:], qv[1], 6, None, SHL)  # u8 wrap
            nc.vector.tensor_tensor(bv[0], qv[0], tA[:], OR)
            nc.vector.tensor_scalar(tA[:], qv[1], 2, None, SHR)
            nc.vector.tensor_scalar(tB[:], qv[2], 4, None, SHL)
            nc.vector.tensor_tensor(bv[1], tA[:], tB[:], OR)
            nc.vector.tensor_scalar(tA[:], qv[2], 4, None, SHR)
            nc.vector.tensor_scalar(tB[:], qv[3], 2, None, SHL)
            nc.vector.tensor_tensor(bv[2], tA[:], tB[:], OR)
            nc.sync.dma_start(out=oout[:, 0:FS6], in_=out_p[:])

    split_excess_waits(nc)
    return nc


def _get_runner():
    """Build nc + the jitted SPMD executor once; cache for warm calls."""
    if "runner" in _CACHE:
        return _CACHE["runner"]
    import jax
    from jax.sharding import Mesh, PartitionSpec
    from jax.experimental.shard_map import shard_map
    from concourse import bass2jax
    import concourse.mybir as mybir

    nc = _build()
    bass2jax.install_neuronx_cc_hook()
    partition_name = (nc.partition_id_tensor.name
                      if nc.partition_id_tensor else None)
    in_names, out_names, out_avals = [], [], []
    for alloc in nc.m.functions[0].allocations:
        if not isinstance(alloc, mybir.MemoryLocationSet):
            continue
        name = alloc.memorylocations[0].name
        if alloc.kind == "ExternalInput":
            if name != partition_name:
                in_names.append(name)
        elif alloc.kind == "ExternalOutput":
            out_names.append(name)
            out_avals.append(jax.core.ShapedArray(
                tuple(alloc.tensor_shape), mybir.dt.np(alloc.dtype)))
    bind_names = tuple(in_names) + ((partition_name,) if partition_name
                                    else ())

    def _body(*args):
        operands = list(args)
        if partition_name is not None:
            operands.append(bass2jax.partition_id_tensor())
        outs = bass2jax._bass_exec_p.bind(
            *operands,
            out_avals=tuple(out_avals),
            in_names=bind_names,
            out_names=tuple(out_names),
            lowering_input_output_aliases=(),
            sim_require_finite=True,
            sim_require_nnan=True,
            nc=nc,
        )
        return tuple(outs)

    devices = jax.devices()[:NCORES]
    fns = []
    for a, b in CHUNKS:
        mesh = Mesh(np.asarray(devices[a:b]), ("core",))
        fns.append(jax.jit(shard_map(
            _body, mesh=mesh,
            in_specs=(PartitionSpec("core"),) * len(in_names),
            out_specs=(PartitionSpec("core"),) * len(out_names),
            check_rep=False)))
    _CACHE["runner"] = (fns, in_names, out_names)
    return _CACHE["runner"]


XB = 2 * 3 * WH                 # bytes of f16 coords per row
CQ = C // 4                     # channel-quads for 6-bit packing
SB = CQ * 3 * WH                # packed softmax bytes per row
ROWB = XB + SB                  # bytes per cin row

# C fast path for the quantize+mask+pack (and output unpack) inner loops;
# compiled on first use, with the numpy implementation as fallback.
_CSRC = r"""
#include <string.h>
#include <stdint.h>

/* sm: (2,20,64,2048) f32, mask: (2,64,2048) i32,
   cin rows (core k, partition p=h*2+n) of 5460 bytes:
   [1560B f16 xyz | 260 pixels * 15B of 6-bit channel quads]  */
void pack_sm(const float* sm, const int32_t* mask, unsigned char* cin,
             int k0, int k1) {
    for (int k = k0; k < k1; ++k)
      for (int h = 0; h < 64; ++h)
        for (int n = 0; n < 2; ++n) {
          unsigned char* row =
              cin + (size_t)((k * 64 + h) * 2 + n) * 5460 + 1560;
          const int32_t* mrow = mask + ((size_t)n * 64 + h) * 2048;
          const float* srow = sm + ((size_t)n * 20 * 64 + h) * 2048;
          for (int j = 0; j < 260; ++j) {
            int col = (k * 256 + j - 2 + 2048) & 2047;
            unsigned char* px = row + (size_t)j * 15;
            if (!mrow[col]) { memset(px, 0, 15); continue; }
            for (int cq = 0; cq < 5; ++cq) {
              const float* s = srow + (size_t)(4 * cq) * 131072 + col;
              unsigned v0 = (unsigned)(s[0] * 63.0f + 0.5f);
              unsigned v1 = (unsigned)(s[131072] * 63.0f + 0.5f);
              unsigned v2 = (unsigned)(s[262144] * 63.0f + 0.5f);
              unsigned v3 = (unsigned)(s[393216] * 63.0f + 0.5f);
              unsigned w = v0 | (v1 << 6) | (v2 << 12) | (v3 << 18);
              px[cq * 3] = w & 255;
              px[cq * 3 + 1] = (w >> 8) & 255;
              px[cq * 3 + 2] = (w >> 16) & 255;
            }
          }
        }
}

/* qk: (128, 3844) u8 shard (6-bit packed + 4B scale), sc: (128,) f32
   dequant scale, out: (2,20,64,2048) f32, writes cols [k*256,(k+1)*256) */
void unpack_out(const unsigned char* qk, const float* sc, float* out,
                int k) {
    for (int p = 0; p < 128; ++p) {
      int h = p >> 1, n = p & 1;
      const unsigned char* q = qk + (size_t)p * 3844;
      float s = sc[p];
      for (int c = 0; c < 20; ++c) {
        float* o = out + (((size_t)n * 20 + c) * 64 + h) * 2048 + k * 256;
        const unsigned char* qq = q + (size_t)c * 192;
        for (int t = 0; t < 64; ++t) {
          unsigned b0 = qq[3 * t], b1 = qq[3 * t + 1], b2 = qq[3 * t + 2];
          o[4 * t]     = (float)(b0 & 63) * s;
          o[4 * t + 1] = (float)((b0 >> 6) | ((b1 & 15) << 2)) * s;
          o[4 * t + 2] = (float)((b1 >> 4) | ((b2 & 3) << 4)) * s;
          o[4 * t + 3] = (float)(b2 >> 2) * s;
        }
      }
    }
}
"""


def _get_clib():
    if "clib" in _CACHE:
        return _CACHE["clib"]
    lib = None
    try:
        import subprocess, tempfile, ctypes, os
        d = tempfile.mkdtemp(prefix="lcxyz_")
        src = os.path.join(d, "pack.c")
        so = os.path.join(d, "pack.so")
        with open(src, "w") as f:
            f.write(_CSRC)
        subprocess.run(["cc", "-O3", "-shared", "-fPIC", src, "-o", so],
                       check=True, capture_output=True, timeout=120)
        lib = ctypes.CDLL(so)
        lib.pack_sm.argtypes = [ctypes.c_void_p] * 3 + [ctypes.c_int] * 2
        lib.pack_sm.restype = None
        lib.unpack_out.argtypes = [ctypes.c_void_p] * 3 + [ctypes.c_int]
        lib.unpack_out.restype = None
    except Exception:
        lib = None
    _CACHE["clib"] = lib
    return lib


def _get_prep_bufs():
    b = _CACHE.get("prep_bufs")
    if b is None:
        from numpy.lib.stride_tricks import as_strided
        cin = np.empty((NCORES * P, ROWB), np.uint8)
        b = _CACHE["prep_bufs"] = {
            "cin": cin,
            "f32": np.empty((N, C, H, W + 4), np.float32),
            "s_e": np.empty((N, C, H, W + 4), np.uint8),
            "x_e": np.empty((N, 3, H, W + 4), np.float16),
            # f16 / u8 views aliasing the packed buffer
            "xv": np.ndarray((NCORES, H, N, 3, WH), np.float16,
                             buffer=cin.data, offset=0,
                             strides=(H * N * ROWB, N * ROWB, ROWB,
                                      2 * WH, 2)),
            # pixel-major: byte (j, cq, t) at j*15 + cq*3 + t so masked
            # pixels are 15-byte zero runs (wire compression eats them)
            "svp": np.ndarray((NCORES, H, N, CQ, WH, 3), np.uint8,
                              buffer=cin.data, offset=XB,
                              strides=(H * N * ROWB, N * ROWB, ROWB,
                                       3, 15, 1)),
        }

        def win_view(a_e):  # (N, CD, H, W+4) -> (8, H, N, CD, WH) view
            t = a_e.transpose(2, 0, 1, 3)
            st = t.strides
            return as_strided(t, shape=(NCORES, H, N, a_e.shape[1], WH),
                              strides=(WC * st[3], st[0], st[1], st[2],
                                       st[3]))

        b["sviews"] = win_view(b["s_e"])
        b["xviews"] = win_view(b["x_e"])
    return b


def _prep_chunk(b, a_core, b_core, xyz, sm, mk, mk32):
    """Quantize/pack only the W-slice needed by cores [a_core, b_core).

    Straight serial numpy/C: this container has a single CPU core, so
    thread pools only add churn.
    """
    x_e = b["x_e"]
    c0 = a_core * WC
    c1 = min(b_core * WC + 2, W)         # +2: right halo of the last core
    if a_core == 0:
        # left wrap halo: ext[...,0:2] = core cols W-2..W
        np.copyto(x_e[..., 0:2], xyz[..., W - 2:W], casting="unsafe")
    np.copyto(x_e[..., c0 + 2:c1 + 2], xyz[..., c0:c1], casting="unsafe")
    if b_core == NCORES:
        # right wrap halo: ext cols W+2..W+4 = core cols 0..2 (from chunk 0)
        x_e[..., W + 2:] = x_e[..., 2:4]
    xv, xviews = b["xv"], b["xviews"]
    for k in range(a_core, b_core):
        np.copyto(xv[k], xviews[k])

    lib = _get_clib()
    if lib is not None:
        lib.pack_sm(sm.ctypes.data, mk32.ctypes.data,
                    b["cin"].ctypes.data, a_core, b_core)
        return
    # ---- numpy fallback: quantize into halo-extended buffer, then pack
    s_e, buf = b["s_e"], b["f32"]
    if a_core == 0:
        bs = buf[..., 0:2]
        np.multiply(sm[..., W - 2:W], 63.0, out=bs)
        bs += 0.5
        np.copyto(s_e[..., 0:2], bs, casting="unsafe")
        s_e[..., 0:2] *= mk[..., W - 2:W]
    bs = buf[..., c0:c1]
    np.multiply(sm[..., c0:c1], 63.0, out=bs)
    bs += 0.5
    core = s_e[..., c0 + 2:c1 + 2]
    np.copyto(core, bs, casting="unsafe")   # f32 -> u8 truncation = astype
    core *= mk[..., c0:c1]
    if b_core == NCORES:
        s_e[..., W + 2:] = s_e[..., 2:4]
    svp, sviews = b["svp"], b["sviews"]
    for k in range(a_core, b_core):
        # 4 channels (one quad) -> little-endian 24-bit group -> 3 bytes,
        # byte-plane u8 math: the wrapping shifts supply the bit masks
        sv = sviews[k]
        v0, v1, v2, v3 = (sv[:, :, 0::4, :], sv[:, :, 1::4, :],
                          sv[:, :, 2::4, :], sv[:, :, 3::4, :])
        svp[k, ..., 0] = v0 | (v1 << 6)          # wrap == (v1 & 3) << 6
        svp[k, ..., 1] = (v1 >> 2) | (v2 << 4)   # wrap == (v2 & 15) << 4
        svp[k, ..., 2] = (v2 >> 4) | (v3 << 2)


def kernel(xyz, softmax, mask):
    fns, in_names, out_names = _get_runner()
    oi = out_names.index("oout")
    xyz = np.asarray(xyz, np.float32)
    sm = np.ascontiguousarray(np.asarray(softmax, np.float32))
    mk32 = np.ascontiguousarray(np.asarray(mask, np.int32))
    lib = _get_clib()
    mk = (None if lib is not None
          else mk32.astype(np.uint8)[:, None])                 # (N,1,H,W)
    b = _get_prep_bufs()
    cin = b["cin"]
    # pipelined: prep + dispatch chunk k, then prep k+1 while k uploads;
    # fetch/dequant chunk k while later chunks are still in flight
    chunk_outs = []
    for ci, (a, bb) in enumerate(CHUNKS):
        _prep_chunk(b, a, bb, xyz, sm, mk, mk32)
        arrs = fns[ci](cin[a * P:bb * P])
        pk = arrs[oi]
        for s in pk.addressable_shards:
            s.data.copy_to_host_async()
        chunk_outs.append(pk)
    out = np.empty((N, C, H, W), np.float32)
    ub = _CACHE.get("unpack_buf")
    if ub is None:
        ub = _CACHE["unpack_buf"] = np.empty((P, NQ, 4), np.uint8)
    for ci, (a, bb) in enumerate(CHUNKS):
        shards = sorted(chunk_outs[ci].addressable_shards,
                        key=lambda s: s.index[0].start or 0)
        for kk, s in enumerate(shards):
            k = a + kk
            qk = np.asarray(s.data)                            # (P, FS6+4) u8
            mx = qk[:, FS6:].copy().view(np.float32)           # (P, 1)
            # dequant: device acc = 63*out_true, q ~= acc * OSCALE/max
            scv = mx * (1.0 / (OSCALE * 63.0))                 # (P, 1) f32
            if lib is not None:
                qkc = qk if qk.flags.c_contiguous else np.ascontiguousarray(qk)
                lib.unpack_out(qkc.ctypes.data, scv.ctypes.data,
                               out.ctypes.data, k)
                continue
            # unpack 3 bytes -> 4 six-bit codes (numpy fallback)
            q3 = qk[:, :FS6].reshape(P, NQ, 3)
            b0, b1, b2 = q3[..., 0], q3[..., 1], q3[..., 2]
            ub[..., 0] = b0 & 63
            ub[..., 1] = (b0 >> 6) | ((b1 & 15) << 2)
            ub[..., 2] = (b1 >> 4) | ((b2 & 3) << 4)
            ub[..., 3] = b2 >> 2
            sc = scv.reshape(H, N)                             # per (i, n)
            np.multiply(
                ub.reshape(P, FS).reshape(H, N, C, WC).transpose(1, 2, 0, 3),
                sc.transpose(1, 0)[:, None, :, None],
                out=out[:, :, :, k * WC:(k + 1) * WC],
                dtype=np.float32)
    return out

